# revision 9
# baseline (speedup 1.0000x reference)
"""Trainium2 Bass kernel: multi-head attention block (nn_Attention_19413252178452).

B=2, N=2048, DIM=2048, H=16, HD=128.
Sharding: tensor-parallel over heads, 2 heads per core across 8 cores.
Each core computes qkv for its heads (w_qkv column-sharded), rmsnorm+rope,
scores/softmax/AV for its heads, and a partial output projection
(w_out row-sharded).  Host sums the 8 partial outputs.

Layout strategy (per core):
  - x is pre-transposed on host: xt [DIM, T] bf16 (T = B*N = 4096).
  - q,k computed directly transposed: qT/kT [HD, T] (head-dim on partitions),
    with head dims permuted (evens then odds) so rope reads contiguous halves.
  - rmsnorm over head dim = partition reduction -> ones-vector matmul on PE;
    rsqrt row broadcast back over partitions via K=1 ones matmul on PE.
  - softmax without max subtraction (scores are O(+-10), exp safe in fp32).
  - scores computed transposed [k_tokens, q_tokens]; row sums of exp via
    ones matmul; AV as vT @ expT -> avT [d, q]; normalization applied to avT.
  - out projection: avT (stationary) @ w_out rows -> y [t, e] accumulated
    over the core's 2 heads in PSUM.
"""

import numpy as np
import ml_dtypes

B, N, DIM = 2, 2048, 2048
H, HD = 16, 128
T = B * N
NCORES = 8
HPC = H // NCORES  # heads per core
EPS = 1e-5
SCALE = 1.0 / float(np.sqrt(HD))

NB = N // 512     # q-chunks of 512 per batch (4)
KT = N // 128     # k-tiles of 128 per batch (16)
NCH = T // 512    # token chunks of 512 total (8)
NTT = T // 128    # token tiles of 128 total (32)
DO = DIM // 128   # dim tiles (16)
NE = DIM // 512   # out-feature chunks of 512 (4)

_CACHE = {}


def _emit(tc, aps):
    import concourse.bass as bass
    import concourse.mybir as mybir
    from contextlib import ExitStack

    dt = mybir.dt
    f32 = dt.float32
    bf16 = dt.bfloat16
    AF = mybir.ActivationFunctionType
    OP = mybir.AluOpType
    nc = tc.nc

    xt, wq, wk, wv, wo, fr, fi, nw, y = (
        aps["xt"], aps["wq"], aps["wk"], aps["wv"], aps["wo"],
        aps["fr"], aps["fi"], aps["nw"], aps["y"],
    )

    ctx = ExitStack()
    with ctx:
        # ---- persistent SBUF pools ----
        persist = ctx.enter_context(tc.tile_pool(name="persist", bufs=1))
        wq_sb = persist.tile([128, DO, HPC * HD], bf16, tag="wq")
        wk_sb = persist.tile([128, DO, HPC * HD], bf16, tag="wk")
        wv_sb = persist.tile([128, DO, HPC * HD], bf16, tag="wv")
        wo_sb = persist.tile([128, HPC, DIM], bf16, tag="wo")
        nw_sb = persist.tile([128, 2], f32, tag="nw")
        ones_col = persist.tile([128, 1], bf16, tag="ones_col")
        ones_bc = persist.tile([1, 128], f32, tag="ones_bc")
        eps_row = persist.tile([1, 1], f32, tag="eps_row")
        qT = [persist.tile([128, T], bf16, tag=f"qT{h}", name=f"qT{h}")
              for h in range(HPC)]
        kT = [persist.tile([128, T], bf16, tag=f"kT{h}", name=f"kT{h}")
              for h in range(HPC)]
        avn = [persist.tile([128, T], bf16, tag=f"avn{h}", name=f"avn{h}")
               for h in range(HPC)]
        v_sb = persist.tile([128, NTT, HPC * HD], bf16, tag="v_sb")

        nc.sync.dma_start(wq_sb[:], wq.rearrange("(o p) f -> p o f", p=128))
        nc.sync.dma_start(wk_sb[:], wk.rearrange("(o p) f -> p o f", p=128))
        nc.sync.dma_start(wv_sb[:], wv.rearrange("(o p) f -> p o f", p=128))
        nc.sync.dma_start(wo_sb[:], wo.rearrange("(h p) e -> p h e", p=128))
        nc.sync.dma_start(nw_sb[:], nw[:, :])
        nc.any.memset(ones_col[:], 1.0)
        nc.any.memset(ones_bc[:], 1.0)
        nc.any.memset(eps_row[:], EPS)

        # ---- working pools ----
        xtp = ctx.enter_context(tc.tile_pool(name="xtp", bufs=2))
        frp = ctx.enter_context(tc.tile_pool(name="frp", bufs=2))
        tmp = ctx.enter_context(tc.tile_pool(name="tmp", bufs=2))
        rows = ctx.enter_context(tc.tile_pool(name="rows", bufs=2))
        bcb = ctx.enter_context(tc.tile_pool(name="bcb", bufs=2))
        expp = ctx.enter_context(tc.tile_pool(name="expp", bufs=4))
        ybp = ctx.enter_context(tc.tile_pool(name="ybp", bufs=3))
        # single PSUM pool for all stages; per-tag buffer counts keep the
        # total at 8 banks while letting stages overlap (no pool-scope
        # barriers between qkv / attention / out-projection).
        psum = ctx.enter_context(tc.tile_pool(name="psum", bufs=2, space="PSUM"))

        def stage_a_chunk(ch):
            t0 = ch * 512
            xts = xtp.tile([128, DO, 512], bf16, tag="xts", name="xts")
            nc.sync.dma_start(
                xts[:], xt[:, t0:t0 + 512].rearrange("(o p) t -> p o t", p=128)
            )
            frt = frp.tile([128, 512], f32, tag="frt", name="frt")
            fit = frp.tile([128, 512], f32, tag="fit", name="fit")
            nc.sync.dma_start(frt[:], fr[:, t0:t0 + 512])
            nc.sync.dma_start(fit[:], fi[:, t0:t0 + 512])

            for h in range(HPC):
                hs = h * HD
                q_ps = psum.tile([128, 512], f32, tag="mm", bufs=4, name="q_ps")
                for o in range(DO):
                    nc.tensor.matmul(
                        q_ps[:], wq_sb[:, o, hs:hs + HD], xts[:, o, :],
                        start=(o == 0), stop=(o == DO - 1),
                    )
                q_sbf = tmp.tile([128, 512], f32, tag="q_sbf", name="q_sbf")
                nc.scalar.copy(q_sbf[:], q_ps[:])

                k_ps = psum.tile([128, 512], f32, tag="mm", bufs=4, name="k_ps")
                for o in range(DO):
                    nc.tensor.matmul(
                        k_ps[:], wk_sb[:, o, hs:hs + HD], xts[:, o, :],
                        start=(o == 0), stop=(o == DO - 1),
                    )
                k_sbf = tmp.tile([128, 512], f32, tag="k_sbf", name="k_sbf")
                nc.scalar.copy(k_sbf[:], k_ps[:])

                for src, src_ps, wcol, dstT in (
                        (q_sbf, q_ps, 0, qT[h]), (k_sbf, k_ps, 1, kT[h])):
                    # half-swapped copy (odds to 0:64, evens to 64:128):
                    # ScalarE permits partition-shifted copies
                    qsw = tmp.tile([128, 512], f32, tag="qsw", name="qsw")
                    nc.scalar.copy(qsw[0:64, :], src_ps[64:128, :])
                    nc.scalar.copy(qsw[64:128, :], src_ps[0:64, :])
                    # sum of squares over head dim (partitions) via PE
                    sq = tmp.tile([128, 512], bf16, tag="sq", name="sq")
                    nc.vector.tensor_mul(sq[:], src[:], src[:])
                    ssq = psum.tile([1, 512], f32, tag="row", bufs=2, name="ssq")
                    nc.tensor.matmul(ssq[:], ones_col[:], sq[:],
                                     start=True, stop=True)
                    sqr = rows.tile([1, 512], f32, tag="sqr", name="sqr")
                    nc.scalar.activation(sqr[:], ssq[:], AF.Sqrt,
                                         bias=eps_row[:], scale=1.0 / HD)
                    rsg = rows.tile([1, 512], f32, tag="rsg", name="rsg")
                    nc.vector.reciprocal(rsg[:], sqr[:])
                    # broadcast rsqrt row to 128 partitions on GPSIMD
                    bc = bcb.tile([128, 512], f32, tag="bc", name="bc")
                    nc.gpsimd.partition_broadcast(bc[:], rsg[:])
                    # rope via duplicated-row freqs (all ops aligned):
                    # m1 = [qe*fr; qo*fr], m2 = [qo*fi; qe*fi]
                    # o_top = m1 - m2 (evens), o_bot = m1 + m2 (odds)
                    m1 = tmp.tile([128, 512], f32, tag="m1", name="m1")
                    m2 = tmp.tile([128, 512], f32, tag="m2", name="m2")
                    ot = tmp.tile([128, 512], f32, tag="ot", name="ot")
                    nc.vector.tensor_mul(m1[:], src[:], frt[:])
                    nc.vector.tensor_mul(m2[:], qsw[:], fit[:])
                    nc.vector.tensor_sub(ot[0:64, :], m1[0:64, :], m2[0:64, :])
                    nc.vector.tensor_add(ot[64:128, :], m1[64:128, :],
                                         m2[64:128, :])
                    # normalize: (ot * norm_w) * rsqrt_bc  -> bf16
                    nc.vector.scalar_tensor_tensor(
                        dstT[:, t0:t0 + 512], ot[:],
                        nw_sb[:, wcol:wcol + 1], bc[:],
                        op0=OP.mult, op1=OP.mult,
                    )

            # v projection (natural layout: [t, d]) for both heads
            for tt in range(4):
                v_ps = psum.tile([128, HPC * HD], f32, tag="acc", bufs=2,
                                 name="v_ps")
                for o in range(DO):
                    nc.tensor.matmul(
                        v_ps[:], xts[:, o, tt * 128:tt * 128 + 128],
                        wv_sb[:, o, :],
                        start=(o == 0), stop=(o == DO - 1),
                    )
                nc.scalar.copy(v_sb[:, ch * 4 + tt, :], v_ps[:])

        def stage_b(b, h):
            boff = b * N
            hs = h * HD
            for qc in range(NB):
                qoff = boff + qc * 512
                av_ps = psum.tile([128, 512], f32, tag="acc", bufs=2,
                                  name="av_ps")
                sums = psum.tile([1, 512], f32, tag="row", bufs=2, name="sums")
                for kt in range(KT):
                    koff = boff + kt * 128
                    sc = psum.tile([128, 512], f32, tag="mm", bufs=4, name="sc")
                    nc.tensor.matmul(
                        sc[:], kT[h][:, koff:koff + 128],
                        qT[h][:, qoff:qoff + 512],
                        start=True, stop=True,
                    )
                    ex = expp.tile([128, 512], bf16, tag="ex", name="ex")
                    nc.scalar.activation(ex[:], sc[:], AF.Exp, scale=SCALE)
                    nc.tensor.matmul(
                        av_ps[:], v_sb[:, b * KT + kt, hs:hs + HD], ex[:],
                        start=(kt == 0), stop=(kt == KT - 1),
                    )
                    nc.tensor.matmul(
                        sums[:], ones_col[:], ex[:],
                        start=(kt == 0), stop=(kt == KT - 1),
                    )
                rcp = rows.tile([1, 512], f32, tag="rcp", name="rcp")
                nc.vector.reciprocal(rcp[:], sums[:])
                bc2 = bcb.tile([128, 512], f32, tag="bc2", name="bc2")
                nc.gpsimd.partition_broadcast(bc2[:], rcp[:])
                nc.vector.tensor_mul(avn[h][:, qoff:qoff + 512],
                                     av_ps[:], bc2[:])

        def stage_c(b, tt):
            toff = b * N + tt * 128
            for ec in range(NE):
                eoff = ec * 512
                y_ps = psum.tile([128, 512], f32, tag="acc", bufs=2, name="y_ps")
                for h in range(HPC):
                    nc.tensor.matmul(
                        y_ps[:], avn[h][:, toff:toff + 128],
                        wo_sb[:, h, eoff:eoff + 512],
                        start=(h == 0), stop=(h == HPC - 1),
                    )
                y_sb = ybp.tile([128, 512], f32, tag="y_sb", name="y_sb")
                nc.vector.tensor_copy(y_sb[:], y_ps[:])
                nc.sync.dma_start(
                    y[toff:toff + 128, eoff:eoff + 512], y_sb[:]
                )

        for b in range(B):
            for ch in range(4 * b, 4 * b + 4):
                stage_a_chunk(ch)
            for h in range(HPC):
                stage_b(b, h)
            for tt in range(KT):
                stage_c(b, tt)


def _build():
    if "nc" in _CACHE:
        return _CACHE["nc"]
    import concourse.bacc as bacc
    import concourse.tile as tile
    import concourse.mybir as mybir

    dt = mybir.dt
    nc = bacc.Bacc(
        "TRN2",
        target_bir_lowering=False,
        debug=False,
        enable_asserts=False,
        num_devices=NCORES,
    )
    aps = {
        "xt": nc.dram_tensor("xt", [DIM, T], dt.bfloat16, kind="ExternalInput").ap(),
        "wq": nc.dram_tensor("wq", [DIM, HPC * HD], dt.bfloat16,
                             kind="ExternalInput").ap(),
        "wk": nc.dram_tensor("wk", [DIM, HPC * HD], dt.bfloat16,
                             kind="ExternalInput").ap(),
        "wv": nc.dram_tensor("wv", [DIM, HPC * HD], dt.bfloat16,
                             kind="ExternalInput").ap(),
        "wo": nc.dram_tensor("wo", [HPC * HD, DIM], dt.bfloat16,
                             kind="ExternalInput").ap(),
        "fr": nc.dram_tensor("fr", [128, T], dt.float32, kind="ExternalInput").ap(),
        "fi": nc.dram_tensor("fi", [128, T], dt.float32, kind="ExternalInput").ap(),
        "nw": nc.dram_tensor("nw", [128, 2], dt.float32, kind="ExternalInput").ap(),
        "y": nc.dram_tensor("y", [T, DIM], dt.float32, kind="ExternalOutput").ap(),
    }
    with tile.TileContext(nc) as tc:
        _emit(tc, aps)
    nc.compile()
    _CACHE["nc"] = nc
    return nc


def _prep_inputs(x, freqs_real, freqs_imag, w_qkv, w_out, q_norm_w, k_norm_w):
    bf = ml_dtypes.bfloat16
    x2 = np.asarray(x, np.float32).reshape(T, DIM)
    xt = np.ascontiguousarray(x2.T).astype(bf)
    frT = np.asarray(freqs_real, np.float32).reshape(T, HD // 2).T
    fiT = np.asarray(freqs_imag, np.float32).reshape(T, HD // 2).T
    frT = np.ascontiguousarray(np.concatenate([frT, frT], axis=0))
    fiT = np.ascontiguousarray(np.concatenate([fiT, fiT], axis=0))
    w_qkv = np.asarray(w_qkv, np.float32)
    w_out = np.asarray(w_out, np.float32)
    qnw = np.asarray(q_norm_w, np.float32)
    knw = np.asarray(k_norm_w, np.float32)

    perm = np.concatenate([np.arange(0, HD, 2), np.arange(1, HD, 2)])
    nw = np.stack([qnw[perm], knw[perm]], axis=1).astype(np.float32)
    nw = np.ascontiguousarray(nw)  # [128, 2]

    in_maps = []
    for c in range(NCORES):
        heads = [c * HPC + j for j in range(HPC)]
        wq_c = np.concatenate(
            [w_qkv[:, h * HD:(h + 1) * HD][:, perm] for h in heads], axis=1)
        wk_c = np.concatenate(
            [w_qkv[:, H * HD + h * HD:H * HD + (h + 1) * HD][:, perm]
             for h in heads], axis=1)
        wv_c = np.concatenate(
            [w_qkv[:, 2 * H * HD + h * HD:2 * H * HD + (h + 1) * HD]
             for h in heads], axis=1)
        wo_c = np.concatenate(
            [w_out[h * HD:(h + 1) * HD, :] for h in heads], axis=0)
        in_maps.append({
            "xt": xt,
            "wq": np.ascontiguousarray(wq_c).astype(bf),
            "wk": np.ascontiguousarray(wk_c).astype(bf),
            "wv": np.ascontiguousarray(wv_c).astype(bf),
            "wo": np.ascontiguousarray(wo_c).astype(bf),
            "fr": frT,
            "fi": fiT,
            "nw": nw,
        })
    return in_maps


def kernel(x, freqs_real, freqs_imag, w_qkv, w_out, q_norm_w, k_norm_w,
           _trace=False):
    from concourse.bass_utils import run_bass_kernel_spmd

    nc = _build()
    in_maps = _prep_inputs(x, freqs_real, freqs_imag, w_qkv, w_out,
                           q_norm_w, k_norm_w)
    res = run_bass_kernel_spmd(
        nc, in_maps, core_ids=list(range(NCORES)),
        trace=_trace, trace_cores=[0] if _trace else None,
    )
    _CACHE["last_result"] = res
    out = np.zeros((T, DIM), np.float64)
    for r in res.results:
        out += r["y"].astype(np.float64)
    return out.astype(np.float32).reshape(B, N, DIM)


# revision 10
# speedup vs baseline: 40.6539x; 40.6539x over previous
"""Trainium2 Bass kernel: multi-head attention block (nn_Attention_19413252178452).

B=2, N=2048, DIM=2048, H=16, HD=128.
Sharding: tensor-parallel over heads, 2 heads per core across 8 cores.
Each core computes qkv for its heads (w_qkv column-sharded), rmsnorm+rope,
scores/softmax/AV for its heads, and a partial output projection
(w_out row-sharded).  Host sums the 8 partial outputs.

Layout strategy (per core):
  - x is pre-transposed on host: xt [DIM, T] bf16 (T = B*N = 4096).
  - q,k computed directly transposed: qT/kT [HD, T] (head-dim on partitions),
    with head dims permuted (evens then odds) so rope reads contiguous halves.
  - rmsnorm over head dim = partition reduction -> ones-vector matmul on PE;
    rsqrt row broadcast back over partitions via K=1 ones matmul on PE.
  - softmax without max subtraction (scores are O(+-10), exp safe in fp32).
  - scores computed transposed [k_tokens, q_tokens]; row sums of exp via
    ones matmul; AV as vT @ expT -> avT [d, q]; normalization applied to avT.
  - out projection: avT (stationary) @ w_out rows -> y [t, e] accumulated
    over the core's 2 heads in PSUM.
"""

import numpy as np
import ml_dtypes

B, N, DIM = 2, 2048, 2048
H, HD = 16, 128
T = B * N
NCORES = 8
HPC = H // NCORES  # heads per core
EPS = 1e-5
SCALE = 1.0 / float(np.sqrt(HD))

NB = N // 512     # q-chunks of 512 per batch (4)
KT = N // 128     # k-tiles of 128 per batch (16)
NCH = T // 512    # token chunks of 512 total (8)
NTT = T // 128    # token tiles of 128 total (32)
DO = DIM // 128   # dim tiles (16)
NE = DIM // 512   # out-feature chunks of 512 (4)

_CACHE = {}


def _emit(tc, aps, repeat=1):
    import concourse.bass as bass
    import concourse.mybir as mybir
    from contextlib import ExitStack

    dt = mybir.dt
    f32 = dt.float32
    bf16 = dt.bfloat16
    AF = mybir.ActivationFunctionType
    OP = mybir.AluOpType
    nc = tc.nc

    xt, wq, wk, wv, wo, fr, fi, nw, y = (
        aps["xt"], aps["wq"], aps["wk"], aps["wv"], aps["wo"],
        aps["fr"], aps["fi"], aps["nw"], aps["y"],
    )

    ctx = ExitStack()
    with ctx:
        # ---- persistent SBUF pools ----
        persist = ctx.enter_context(tc.tile_pool(name="persist", bufs=1))
        wq_sb = persist.tile([128, DO, HPC * HD], bf16, tag="wq")
        wk_sb = persist.tile([128, DO, HPC * HD], bf16, tag="wk")
        wv_sb = persist.tile([128, DO, HPC * HD], bf16, tag="wv")
        wo_sb = persist.tile([128, HPC, DIM], bf16, tag="wo")
        nw_sb = persist.tile([128, 2], f32, tag="nw")
        ones_col = persist.tile([128, 1], bf16, tag="ones_col")
        ones_bc = persist.tile([1, 128], f32, tag="ones_bc")
        eps_row = persist.tile([1, 1], f32, tag="eps_row")
        qT = [persist.tile([128, T], bf16, tag=f"qT{h}", name=f"qT{h}")
              for h in range(HPC)]
        kT = [persist.tile([128, T], bf16, tag=f"kT{h}", name=f"kT{h}")
              for h in range(HPC)]
        avn = [persist.tile([128, T], bf16, tag=f"avn{h}", name=f"avn{h}")
               for h in range(HPC)]
        v_sb = persist.tile([128, NTT, HPC * HD], bf16, tag="v_sb")

        nc.sync.dma_start(wq_sb[:], wq.rearrange("(o p) f -> p o f", p=128))
        nc.sync.dma_start(wk_sb[:], wk.rearrange("(o p) f -> p o f", p=128))
        nc.sync.dma_start(wv_sb[:], wv.rearrange("(o p) f -> p o f", p=128))
        nc.sync.dma_start(wo_sb[:], wo.rearrange("(h p) e -> p h e", p=128))
        nc.sync.dma_start(nw_sb[:], nw[:, :])
        nc.any.memset(ones_col[:], 1.0)
        nc.any.memset(ones_bc[:], 1.0)
        nc.any.memset(eps_row[:], EPS)

        # ---- working pools ----
        xtp = ctx.enter_context(tc.tile_pool(name="xtp", bufs=2))
        frp = ctx.enter_context(tc.tile_pool(name="frp", bufs=2))
        tmp = ctx.enter_context(tc.tile_pool(name="tmp", bufs=2))
        rows = ctx.enter_context(tc.tile_pool(name="rows", bufs=2))
        bcb = ctx.enter_context(tc.tile_pool(name="bcb", bufs=2))
        expp = ctx.enter_context(tc.tile_pool(name="expp", bufs=4))
        ybp = ctx.enter_context(tc.tile_pool(name="ybp", bufs=3))
        # single PSUM pool for all stages; per-tag buffer counts keep the
        # total at 8 banks while letting stages overlap (no pool-scope
        # barriers between qkv / attention / out-projection).
        psum = ctx.enter_context(tc.tile_pool(name="psum", bufs=2, space="PSUM"))

        def stage_a_chunk(ch):
            t0 = ch * 512
            xts = xtp.tile([128, DO, 512], bf16, tag="xts", name="xts")
            nc.sync.dma_start(
                xts[:], xt[:, t0:t0 + 512].rearrange("(o p) t -> p o t", p=128)
            )
            frt = frp.tile([128, 512], f32, tag="frt", name="frt")
            fit = frp.tile([128, 512], f32, tag="fit", name="fit")
            nc.sync.dma_start(frt[:], fr[:, t0:t0 + 512])
            nc.sync.dma_start(fit[:], fi[:, t0:t0 + 512])

            for h in range(HPC):
                hs = h * HD
                q_ps = psum.tile([128, 512], f32, tag="mm", bufs=4, name="q_ps")
                for o in range(DO):
                    nc.tensor.matmul(
                        q_ps[:], wq_sb[:, o, hs:hs + HD], xts[:, o, :],
                        start=(o == 0), stop=(o == DO - 1),
                    )
                q_sbf = tmp.tile([128, 512], f32, tag="q_sbf", name="q_sbf")
                nc.scalar.copy(q_sbf[:], q_ps[:])

                k_ps = psum.tile([128, 512], f32, tag="mm", bufs=4, name="k_ps")
                for o in range(DO):
                    nc.tensor.matmul(
                        k_ps[:], wk_sb[:, o, hs:hs + HD], xts[:, o, :],
                        start=(o == 0), stop=(o == DO - 1),
                    )
                k_sbf = tmp.tile([128, 512], f32, tag="k_sbf", name="k_sbf")
                nc.scalar.copy(k_sbf[:], k_ps[:])

                for src, src_ps, wcol, dstT in (
                        (q_sbf, q_ps, 0, qT[h]), (k_sbf, k_ps, 1, kT[h])):
                    # half-swapped copy (odds to 0:64, evens to 64:128):
                    # ScalarE permits partition-shifted copies
                    qsw = tmp.tile([128, 512], f32, tag="qsw", name="qsw")
                    nc.scalar.copy(qsw[0:64, :], src_ps[64:128, :])
                    nc.scalar.copy(qsw[64:128, :], src_ps[0:64, :])
                    # sum of squares over head dim (partitions) via PE
                    sq = tmp.tile([128, 512], bf16, tag="sq", name="sq")
                    nc.vector.tensor_mul(sq[:], src[:], src[:])
                    ssq = psum.tile([1, 512], f32, tag="row", bufs=2, name="ssq")
                    nc.tensor.matmul(ssq[:], ones_col[:], sq[:],
                                     start=True, stop=True)
                    sqr = rows.tile([1, 512], f32, tag="sqr", name="sqr")
                    nc.scalar.activation(sqr[:], ssq[:], AF.Sqrt,
                                         bias=eps_row[:], scale=1.0 / HD)
                    rsg = rows.tile([1, 512], f32, tag="rsg", name="rsg")
                    nc.vector.reciprocal(rsg[:], sqr[:])
                    # broadcast rsqrt row to 128 partitions on GPSIMD
                    bc = bcb.tile([128, 512], f32, tag="bc", name="bc")
                    nc.gpsimd.partition_broadcast(bc[:], rsg[:])
                    # rope via duplicated-row freqs (all ops aligned):
                    # m1 = [qe*fr; qo*fr], m2 = [qo*fi; qe*fi]
                    # o_top = m1 - m2 (evens), o_bot = m1 + m2 (odds)
                    m1 = tmp.tile([128, 512], f32, tag="m1", name="m1")
                    m2 = tmp.tile([128, 512], f32, tag="m2", name="m2")
                    ot = tmp.tile([128, 512], f32, tag="ot", name="ot")
                    nc.vector.tensor_mul(m1[:], src[:], frt[:])
                    nc.vector.tensor_mul(m2[:], qsw[:], fit[:])
                    nc.vector.tensor_sub(ot[0:64, :], m1[0:64, :], m2[0:64, :])
                    nc.vector.tensor_add(ot[64:128, :], m1[64:128, :],
                                         m2[64:128, :])
                    # normalize: (ot * norm_w) * rsqrt_bc  -> bf16
                    nc.vector.scalar_tensor_tensor(
                        dstT[:, t0:t0 + 512], ot[:],
                        nw_sb[:, wcol:wcol + 1], bc[:],
                        op0=OP.mult, op1=OP.mult,
                    )

            # v projection (natural layout: [t, d]) for both heads
            for tt in range(4):
                v_ps = psum.tile([128, HPC * HD], f32, tag="acc", bufs=2,
                                 name="v_ps")
                for o in range(DO):
                    nc.tensor.matmul(
                        v_ps[:], xts[:, o, tt * 128:tt * 128 + 128],
                        wv_sb[:, o, :],
                        start=(o == 0), stop=(o == DO - 1),
                    )
                nc.scalar.copy(v_sb[:, ch * 4 + tt, :], v_ps[:])

        def stage_b(b, h):
            boff = b * N
            hs = h * HD
            for qc in range(NB):
                qoff = boff + qc * 512
                av_ps = psum.tile([128, 512], f32, tag="acc", bufs=2,
                                  name="av_ps")
                sums = psum.tile([1, 512], f32, tag="row", bufs=2, name="sums")
                for kt in range(KT):
                    koff = boff + kt * 128
                    sc = psum.tile([128, 512], f32, tag="mm", bufs=4, name="sc")
                    nc.tensor.matmul(
                        sc[:], kT[h][:, koff:koff + 128],
                        qT[h][:, qoff:qoff + 512],
                        start=True, stop=True,
                    )
                    ex = expp.tile([128, 512], bf16, tag="ex", name="ex")
                    nc.scalar.activation(ex[:], sc[:], AF.Exp, scale=SCALE)
                    nc.tensor.matmul(
                        av_ps[:], v_sb[:, b * KT + kt, hs:hs + HD], ex[:],
                        start=(kt == 0), stop=(kt == KT - 1),
                    )
                    nc.tensor.matmul(
                        sums[:], ones_col[:], ex[:],
                        start=(kt == 0), stop=(kt == KT - 1),
                    )
                rcp = rows.tile([1, 512], f32, tag="rcp", name="rcp")
                nc.vector.reciprocal(rcp[:], sums[:])
                bc2 = bcb.tile([128, 512], f32, tag="bc2", name="bc2")
                nc.gpsimd.partition_broadcast(bc2[:], rcp[:])
                nc.vector.tensor_mul(avn[h][:, qoff:qoff + 512],
                                     av_ps[:], bc2[:])

        def stage_c(b, tt):
            toff = b * N + tt * 128
            for ec in range(NE):
                eoff = ec * 512
                y_ps = psum.tile([128, 512], f32, tag="acc", bufs=2, name="y_ps")
                for h in range(HPC):
                    nc.tensor.matmul(
                        y_ps[:], avn[h][:, toff:toff + 128],
                        wo_sb[:, h, eoff:eoff + 512],
                        start=(h == 0), stop=(h == HPC - 1),
                    )
                y_sb = ybp.tile([128, 512], f32, tag="y_sb", name="y_sb")
                nc.vector.tensor_copy(y_sb[:], y_ps[:])
                nc.sync.dma_start(
                    y[toff:toff + 128, eoff:eoff + 512], y_sb[:]
                )

        for _rep in range(repeat):
            for b in range(B):
                for ch in range(4 * b, 4 * b + 4):
                    stage_a_chunk(ch)
                for h in range(HPC):
                    stage_b(b, h)
                for tt in range(KT):
                    stage_c(b, tt)


def _build(repeat=1):
    key = f"nc{repeat}"
    if key in _CACHE:
        return _CACHE[key]
    import concourse.bacc as bacc
    import concourse.tile as tile
    import concourse.mybir as mybir

    dt = mybir.dt
    nc = bacc.Bacc(
        "TRN2",
        target_bir_lowering=False,
        debug=False,
        enable_asserts=False,
        num_devices=NCORES,
    )
    aps = {
        "xt": nc.dram_tensor("xt", [DIM, T], dt.bfloat16, kind="ExternalInput").ap(),
        "wq": nc.dram_tensor("wq", [DIM, HPC * HD], dt.bfloat16,
                             kind="ExternalInput").ap(),
        "wk": nc.dram_tensor("wk", [DIM, HPC * HD], dt.bfloat16,
                             kind="ExternalInput").ap(),
        "wv": nc.dram_tensor("wv", [DIM, HPC * HD], dt.bfloat16,
                             kind="ExternalInput").ap(),
        "wo": nc.dram_tensor("wo", [HPC * HD, DIM], dt.bfloat16,
                             kind="ExternalInput").ap(),
        "fr": nc.dram_tensor("fr", [128, T], dt.float32, kind="ExternalInput").ap(),
        "fi": nc.dram_tensor("fi", [128, T], dt.float32, kind="ExternalInput").ap(),
        "nw": nc.dram_tensor("nw", [128, 2], dt.float32, kind="ExternalInput").ap(),
        "y": nc.dram_tensor("y", [T, DIM], dt.float32, kind="ExternalOutput").ap(),
    }
    with tile.TileContext(nc) as tc:
        _emit(tc, aps, repeat=repeat)
    nc.compile()
    _CACHE[key] = nc
    return nc


def _prep_inputs(x, freqs_real, freqs_imag, w_qkv, w_out, q_norm_w, k_norm_w):
    bf = ml_dtypes.bfloat16
    x2 = np.asarray(x, np.float32).reshape(T, DIM)
    xt = np.ascontiguousarray(x2.T).astype(bf)
    frT = np.asarray(freqs_real, np.float32).reshape(T, HD // 2).T
    fiT = np.asarray(freqs_imag, np.float32).reshape(T, HD // 2).T
    frT = np.ascontiguousarray(np.concatenate([frT, frT], axis=0))
    fiT = np.ascontiguousarray(np.concatenate([fiT, fiT], axis=0))
    w_qkv = np.asarray(w_qkv, np.float32)
    w_out = np.asarray(w_out, np.float32)
    qnw = np.asarray(q_norm_w, np.float32)
    knw = np.asarray(k_norm_w, np.float32)

    perm = np.concatenate([np.arange(0, HD, 2), np.arange(1, HD, 2)])
    nw = np.stack([qnw[perm], knw[perm]], axis=1).astype(np.float32)
    nw = np.ascontiguousarray(nw)  # [128, 2]

    in_maps = []
    for c in range(NCORES):
        heads = [c * HPC + j for j in range(HPC)]
        wq_c = np.concatenate(
            [w_qkv[:, h * HD:(h + 1) * HD][:, perm] for h in heads], axis=1)
        wk_c = np.concatenate(
            [w_qkv[:, H * HD + h * HD:H * HD + (h + 1) * HD][:, perm]
             for h in heads], axis=1)
        wv_c = np.concatenate(
            [w_qkv[:, 2 * H * HD + h * HD:2 * H * HD + (h + 1) * HD]
             for h in heads], axis=1)
        wo_c = np.concatenate(
            [w_out[h * HD:(h + 1) * HD, :] for h in heads], axis=0)
        in_maps.append({
            "xt": xt,
            "wq": np.ascontiguousarray(wq_c).astype(bf),
            "wk": np.ascontiguousarray(wk_c).astype(bf),
            "wv": np.ascontiguousarray(wv_c).astype(bf),
            "wo": np.ascontiguousarray(wo_c).astype(bf),
            "fr": frT,
            "fi": fiT,
            "nw": nw,
        })
    return in_maps


def kernel(x, freqs_real, freqs_imag, w_qkv, w_out, q_norm_w, k_norm_w,
           _trace=False):
    from concourse.bass_utils import run_bass_kernel_spmd

    nc = _build()
    in_maps = _prep_inputs(x, freqs_real, freqs_imag, w_qkv, w_out,
                           q_norm_w, k_norm_w)
    res = run_bass_kernel_spmd(
        nc, in_maps, core_ids=list(range(NCORES)),
        trace=_trace, trace_cores=[0] if _trace else None,
    )
    _CACHE["last_result"] = res
    out = np.zeros((T, DIM), np.float64)
    for r in res.results:
        out += r["y"].astype(np.float64)
    return out.astype(np.float32).reshape(B, N, DIM)


# revision 14
# speedup vs baseline: 171.1836x; 4.2107x over previous
"""Trainium2 Bass kernel: multi-head attention block (nn_Attention_19413252178452).

B=2, N=2048, DIM=2048, H=16, HD=128.
Sharding: tensor-parallel over heads, 2 heads per core across 8 cores.
Each core computes qkv for its heads (w_qkv column-sharded), rmsnorm+rope,
scores/softmax/AV for its heads, and a partial output projection
(w_out row-sharded).  Host sums the 8 partial outputs.

Layout strategy (per core):
  - x is pre-transposed on host: xt [DIM, T] bf16 (T = B*N = 4096).
  - q,k computed directly transposed: qT/kT [HD, T] (head-dim on partitions),
    with head dims permuted (evens then odds) so rope reads contiguous halves.
  - rmsnorm over head dim = partition reduction -> ones-vector matmul on PE;
    rsqrt row broadcast back over partitions via K=1 ones matmul on PE.
  - softmax without max subtraction (scores are O(+-10), exp safe in fp32).
  - scores computed transposed [k_tokens, q_tokens]; row sums of exp via
    ones matmul; AV as vT @ expT -> avT [d, q]; normalization applied to avT.
  - out projection: avT (stationary) @ w_out rows -> y [t, e] accumulated
    over the core's 2 heads in PSUM.
"""

import numpy as np
import ml_dtypes

B, N, DIM = 2, 2048, 2048
H, HD = 16, 128
T = B * N
NCORES = 8
HPC = H // NCORES  # heads per core
EPS = 1e-5
SCALE = 1.0 / float(np.sqrt(HD))

NB = N // 512     # q-chunks of 512 per batch (4)
KT = N // 128     # k-tiles of 128 per batch (16)
NCH = T // 512    # token chunks of 512 total (8)
NTT = T // 128    # token tiles of 128 total (32)
DO = DIM // 128   # dim tiles (16)
NE = DIM // 512   # out-feature chunks of 512 (4)

_CACHE = {}


def _emit(tc, aps, repeat=1):
    import concourse.bass as bass
    import concourse.mybir as mybir
    from contextlib import ExitStack

    dt = mybir.dt
    f32 = dt.float32
    bf16 = dt.bfloat16
    AF = mybir.ActivationFunctionType
    OP = mybir.AluOpType
    nc = tc.nc

    xt, wq, wk, wv, wo, fr, fi, nw, y = (
        aps["xt"], aps["wq"], aps["wk"], aps["wv"], aps["wo"],
        aps["fr"], aps["fi"], aps["nw"], aps["y"],
    )

    ctx = ExitStack()
    with ctx:
        # ---- persistent SBUF pools ----
        persist = ctx.enter_context(tc.tile_pool(name="persist", bufs=1))
        wq_sb = persist.tile([128, DO, HPC * HD], bf16, tag="wq")
        wk_sb = persist.tile([128, DO, HPC * HD], bf16, tag="wk")
        wv_sb = persist.tile([128, DO, HPC * HD], bf16, tag="wv")
        wo_sb = persist.tile([128, HPC, DIM], bf16, tag="wo")
        nw_sb = persist.tile([128, 2], f32, tag="nw")
        ones_col = persist.tile([128, 1], bf16, tag="ones_col")
        ones_bc = persist.tile([1, 128], f32, tag="ones_bc")
        eps_row = persist.tile([1, 1], f32, tag="eps_row")
        qT = [persist.tile([128, T], bf16, tag=f"qT{h}", name=f"qT{h}")
              for h in range(HPC)]
        kT = [persist.tile([128, T], bf16, tag=f"kT{h}", name=f"kT{h}")
              for h in range(HPC)]
        avn = [persist.tile([128, T], bf16, tag=f"avn{h}", name=f"avn{h}")
               for h in range(HPC)]
        v_sb = persist.tile([128, NTT, HPC * HD], bf16, tag="v_sb")

        for og in range(4):
            osl = slice(og * (DO // 4) * 128, (og + 1) * (DO // 4) * 128)
            otl = slice(og * (DO // 4), (og + 1) * (DO // 4))
            nc.sync.dma_start(
                wq_sb[:, otl, :],
                wq[osl, :].rearrange("(o p) f -> p o f", p=128))
            nc.sync.dma_start(
                wk_sb[:, otl, :],
                wk[osl, :].rearrange("(o p) f -> p o f", p=128))
            nc.sync.dma_start(
                wv_sb[:, otl, :],
                wv[osl, :].rearrange("(o p) f -> p o f", p=128))
        nc.sync.dma_start(wo_sb[:], wo.rearrange("(h p) e -> p h e", p=128))
        nc.sync.dma_start(nw_sb[:], nw[:, :])
        nc.any.memset(ones_col[:], 1.0)
        nc.any.memset(ones_bc[:], 1.0)
        nc.any.memset(eps_row[:], EPS)

        # ---- working pools ----
        xtp = ctx.enter_context(tc.tile_pool(name="xtp", bufs=2))
        frp = ctx.enter_context(tc.tile_pool(name="frp", bufs=2))
        tmp = ctx.enter_context(tc.tile_pool(name="tmp", bufs=2))
        rows = ctx.enter_context(tc.tile_pool(name="rows", bufs=2))
        bcb = ctx.enter_context(tc.tile_pool(name="bcb", bufs=2))
        expp = ctx.enter_context(tc.tile_pool(name="expp", bufs=4))
        ybp = ctx.enter_context(tc.tile_pool(name="ybp", bufs=6))
        # single PSUM pool for all stages; per-tag buffer counts keep the
        # total at 8 banks while letting stages overlap (no pool-scope
        # barriers between qkv / attention / out-projection).
        psum = ctx.enter_context(tc.tile_pool(name="psum", bufs=2, space="PSUM"))

        def stage_a_chunk(ch):
            t0 = ch * 512
            xts = xtp.tile([128, DO, 512], bf16, tag="xts", name="xts")
            for og in range(4):
                osl = slice(og * (DO // 4) * 128, (og + 1) * (DO // 4) * 128)
                otl = slice(og * (DO // 4), (og + 1) * (DO // 4))
                nc.scalar.dma_start(
                    xts[:, otl, :],
                    xt[osl, t0:t0 + 512].rearrange("(o p) t -> p o t", p=128))
            frt = frp.tile([128, 512], f32, tag="frt", name="frt")
            fit = frp.tile([128, 512], f32, tag="fit", name="fit")
            nc.gpsimd.dma_start(frt[:], fr[:, t0:t0 + 512])
            nc.gpsimd.dma_start(fit[:], fi[:, t0:t0 + 512])

            for h in range(HPC):
                hs = h * HD
                q_ps = psum.tile([128, 512], f32, tag="mm", bufs=3, name="q_ps")
                for o in range(DO):
                    nc.tensor.matmul(
                        q_ps[:], wq_sb[:, o, hs:hs + HD], xts[:, o, :],
                        start=(o == 0), stop=(o == DO - 1),
                    )
                q_sbf = tmp.tile([128, 512], f32, tag="q_sbf", name="q_sbf")
                nc.scalar.copy(q_sbf[:], q_ps[:])

                k_ps = psum.tile([128, 512], f32, tag="mm", bufs=3, name="k_ps")
                for o in range(DO):
                    nc.tensor.matmul(
                        k_ps[:], wk_sb[:, o, hs:hs + HD], xts[:, o, :],
                        start=(o == 0), stop=(o == DO - 1),
                    )
                k_sbf = tmp.tile([128, 512], f32, tag="k_sbf", name="k_sbf")
                nc.scalar.copy(k_sbf[:], k_ps[:])

                for src, src_ps, wcol, dstT in (
                        (q_sbf, q_ps, 0, qT[h]), (k_sbf, k_ps, 1, kT[h])):
                    # half-swapped copy (odds to 0:64, evens to 64:128):
                    # ScalarE permits partition-shifted copies
                    qsw = tmp.tile([128, 512], f32, tag="qsw", name="qsw")
                    nc.scalar.copy(qsw[0:64, :], src_ps[64:128, :])
                    nc.scalar.copy(qsw[64:128, :], src_ps[0:64, :])
                    # sum of squares over head dim (partitions) via PE
                    sq = tmp.tile([128, 512], bf16, tag="sq", name="sq")
                    nc.vector.tensor_mul(sq[:], src[:], src[:])
                    ssq = psum.tile([1, 512], f32, tag="row", bufs=2, name="ssq")
                    nc.tensor.matmul(ssq[:], ones_col[:], sq[:],
                                     start=True, stop=True)
                    sqr = rows.tile([1, 512], f32, tag="sqr", name="sqr")
                    nc.scalar.activation(sqr[:], ssq[:], AF.Sqrt,
                                         bias=eps_row[:], scale=1.0 / HD)
                    rsg = rows.tile([1, 512], f32, tag="rsg", name="rsg")
                    nc.vector.reciprocal(rsg[:], sqr[:])
                    # broadcast rsqrt row to 128 partitions on GPSIMD
                    bc = bcb.tile([128, 512], f32, tag="bc", name="bc")
                    nc.gpsimd.partition_broadcast(bc[:], rsg[:])
                    # rope via duplicated-row freqs (all ops aligned):
                    # m1 = [qe*fr; qo*fr], m2 = [qo*fi; qe*fi]
                    # o_top = m1 - m2 (evens), o_bot = m1 + m2 (odds)
                    m1 = tmp.tile([128, 512], f32, tag="m1", name="m1")
                    m2 = tmp.tile([128, 512], f32, tag="m2", name="m2")
                    ot = tmp.tile([128, 512], f32, tag="ot", name="ot")
                    nc.vector.tensor_mul(m1[:], src[:], frt[:])
                    nc.vector.tensor_mul(m2[:], qsw[:], fit[:])
                    nc.vector.tensor_sub(ot[0:64, :], m1[0:64, :], m2[0:64, :])
                    nc.vector.tensor_add(ot[64:128, :], m1[64:128, :],
                                         m2[64:128, :])
                    # normalize: (ot * norm_w) * rsqrt_bc  -> bf16
                    nc.vector.scalar_tensor_tensor(
                        dstT[:, t0:t0 + 512], ot[:],
                        nw_sb[:, wcol:wcol + 1], bc[:],
                        op0=OP.mult, op1=OP.mult,
                    )

            # v projection (natural layout: [t, d]) for both heads
            for tt in range(4):
                v_ps = psum.tile([128, HPC * HD], f32, tag="acc", bufs=3,
                                 name="v_ps")
                for o in range(DO):
                    nc.tensor.matmul(
                        v_ps[:], xts[:, o, tt * 128:tt * 128 + 128],
                        wv_sb[:, o, :],
                        start=(o == 0), stop=(o == DO - 1),
                    )
                nc.scalar.copy(v_sb[:, ch * 4 + tt, :], v_ps[:])

        def stage_b(b, h, qc):
            boff = b * N
            hs = h * HD
            if True:
                qoff = boff + qc * 512
                av_ps = psum.tile([128, 512], f32, tag="acc", bufs=3,
                                  name="av_ps")
                sums = psum.tile([1, 512], f32, tag="row", bufs=2, name="sums")
                for kt in range(KT):
                    koff = boff + kt * 128
                    sc = psum.tile([128, 512], f32, tag="mm", bufs=3, name="sc")
                    nc.tensor.matmul(
                        sc[:], kT[h][:, koff:koff + 128],
                        qT[h][:, qoff:qoff + 512],
                        start=True, stop=True,
                    )
                    ex = expp.tile([128, 512], bf16, tag="ex", name="ex")
                    nc.scalar.activation(ex[:], sc[:], AF.Exp, scale=SCALE)
                    nc.tensor.matmul(
                        av_ps[:], v_sb[:, b * KT + kt, hs:hs + HD], ex[:],
                        start=(kt == 0), stop=(kt == KT - 1),
                    )
                    nc.tensor.matmul(
                        sums[:], ones_col[:], ex[:],
                        start=(kt == 0), stop=(kt == KT - 1),
                    )
                rcp = rows.tile([1, 512], f32, tag="rcp", name="rcp")
                nc.vector.reciprocal(rcp[:], sums[:])
                bc2 = bcb.tile([128, 512], f32, tag="bc2", name="bc2")
                nc.gpsimd.partition_broadcast(bc2[:], rcp[:])
                nc.vector.tensor_mul(avn[h][:, qoff:qoff + 512],
                                     av_ps[:], bc2[:])

        def stage_c(b, tt):
            toff = b * N + tt * 128
            for ec in range(NE):
                eoff = ec * 512
                y_ps = psum.tile([128, 512], f32, tag="acc", bufs=3, name="y_ps")
                for h in range(HPC):
                    nc.tensor.matmul(
                        y_ps[:], avn[h][:, toff:toff + 128],
                        wo_sb[:, h, eoff:eoff + 512],
                        start=(h == 0), stop=(h == HPC - 1),
                    )
                y_sb = ybp.tile([128, 512], f32, tag="y_sb", name="y_sb")
                if ec % 2 == 0:
                    nc.vector.tensor_copy(y_sb[:], y_ps[:])
                else:
                    nc.scalar.copy(y_sb[:], y_ps[:])
                dma_eng = nc.sync if ec % 2 == 0 else nc.scalar
                dma_eng.dma_start(
                    y[toff:toff + 128, eoff:eoff + 512], y_sb[:]
                )

        for _rep in range(repeat):
            for b in range(B):
                for ch in range(4 * b, 4 * b + 4):
                    stage_a_chunk(ch)
                for h in range(HPC):
                    for qc in range(NB):
                        stage_b(b, h, qc)
                for tt in range(KT):
                    stage_c(b, tt)


def _build(repeat=1):
    key = f"nc{repeat}"
    if key in _CACHE:
        return _CACHE[key]
    import concourse.bacc as bacc
    import concourse.tile as tile
    import concourse.mybir as mybir

    dt = mybir.dt
    nc = bacc.Bacc(
        "TRN2",
        target_bir_lowering=False,
        debug=False,
        enable_asserts=False,
        num_devices=NCORES,
    )
    aps = {
        "xt": nc.dram_tensor("xt", [DIM, T], dt.bfloat16, kind="ExternalInput").ap(),
        "wq": nc.dram_tensor("wq", [DIM, HPC * HD], dt.bfloat16,
                             kind="ExternalInput").ap(),
        "wk": nc.dram_tensor("wk", [DIM, HPC * HD], dt.bfloat16,
                             kind="ExternalInput").ap(),
        "wv": nc.dram_tensor("wv", [DIM, HPC * HD], dt.bfloat16,
                             kind="ExternalInput").ap(),
        "wo": nc.dram_tensor("wo", [HPC * HD, DIM], dt.bfloat16,
                             kind="ExternalInput").ap(),
        "fr": nc.dram_tensor("fr", [128, T], dt.float32, kind="ExternalInput").ap(),
        "fi": nc.dram_tensor("fi", [128, T], dt.float32, kind="ExternalInput").ap(),
        "nw": nc.dram_tensor("nw", [128, 2], dt.float32, kind="ExternalInput").ap(),
        "y": nc.dram_tensor("y", [T, DIM], dt.float32, kind="ExternalOutput").ap(),
    }
    with tile.TileContext(nc) as tc:
        _emit(tc, aps, repeat=repeat)
    nc.compile()
    _CACHE[key] = nc
    return nc


def _prep_inputs(x, freqs_real, freqs_imag, w_qkv, w_out, q_norm_w, k_norm_w):
    bf = ml_dtypes.bfloat16
    x2 = np.asarray(x, np.float32).reshape(T, DIM)
    xt = np.ascontiguousarray(x2.T).astype(bf)
    frT = np.asarray(freqs_real, np.float32).reshape(T, HD // 2).T
    fiT = np.asarray(freqs_imag, np.float32).reshape(T, HD // 2).T
    frT = np.ascontiguousarray(np.concatenate([frT, frT], axis=0))
    fiT = np.ascontiguousarray(np.concatenate([fiT, fiT], axis=0))
    w_qkv = np.asarray(w_qkv, np.float32)
    w_out = np.asarray(w_out, np.float32)
    qnw = np.asarray(q_norm_w, np.float32)
    knw = np.asarray(k_norm_w, np.float32)

    perm = np.concatenate([np.arange(0, HD, 2), np.arange(1, HD, 2)])
    nw = np.stack([qnw[perm], knw[perm]], axis=1).astype(np.float32)
    nw = np.ascontiguousarray(nw)  # [128, 2]

    in_maps = []
    for c in range(NCORES):
        heads = [c * HPC + j for j in range(HPC)]
        wq_c = np.concatenate(
            [w_qkv[:, h * HD:(h + 1) * HD][:, perm] for h in heads], axis=1)
        wk_c = np.concatenate(
            [w_qkv[:, H * HD + h * HD:H * HD + (h + 1) * HD][:, perm]
             for h in heads], axis=1)
        wv_c = np.concatenate(
            [w_qkv[:, 2 * H * HD + h * HD:2 * H * HD + (h + 1) * HD]
             for h in heads], axis=1)
        wo_c = np.concatenate(
            [w_out[h * HD:(h + 1) * HD, :] for h in heads], axis=0)
        in_maps.append({
            "xt": xt,
            "wq": np.ascontiguousarray(wq_c).astype(bf),
            "wk": np.ascontiguousarray(wk_c).astype(bf),
            "wv": np.ascontiguousarray(wv_c).astype(bf),
            "wo": np.ascontiguousarray(wo_c).astype(bf),
            "fr": frT,
            "fi": fiT,
            "nw": nw,
        })
    return in_maps


def kernel(x, freqs_real, freqs_imag, w_qkv, w_out, q_norm_w, k_norm_w,
           _trace=False):
    from concourse.bass_utils import run_bass_kernel_spmd

    nc = _build()
    in_maps = _prep_inputs(x, freqs_real, freqs_imag, w_qkv, w_out,
                           q_norm_w, k_norm_w)
    res = run_bass_kernel_spmd(
        nc, in_maps, core_ids=list(range(NCORES)),
        trace=_trace, trace_cores=[0] if _trace else None,
    )
    _CACHE["last_result"] = res
    out = np.zeros((T, DIM), np.float64)
    for r in res.results:
        out += r["y"].astype(np.float64)
    return out.astype(np.float32).reshape(B, N, DIM)


# revision 17
# speedup vs baseline: 207.8560x; 1.2142x over previous
"""Trainium2 Bass kernel: multi-head attention block (nn_Attention_19413252178452).

B=2, N=2048, DIM=2048, H=16, HD=128.
Sharding: tensor-parallel over heads, 2 heads per core across 8 cores.
Each core computes qkv for its heads (w_qkv column-sharded), rmsnorm+rope,
scores/softmax/AV for its heads, and a partial output projection
(w_out row-sharded).  Host sums the 8 partial outputs.

Layout strategy (per core):
  - x is pre-transposed on host: xt [DIM, T] bf16 (T = B*N = 4096).
  - q,k computed directly transposed: qT/kT [HD, T] (head-dim on partitions),
    with head dims permuted (evens then odds) so rope reads contiguous halves.
  - rmsnorm over head dim = partition reduction -> ones-vector matmul on PE;
    rsqrt row broadcast back over partitions via K=1 ones matmul on PE.
  - softmax without max subtraction (scores are O(+-10), exp safe in fp32).
  - scores computed transposed [k_tokens, q_tokens]; row sums of exp via
    ones matmul; AV as vT @ expT -> avT [d, q]; normalization applied to avT.
  - out projection: avT (stationary) @ w_out rows -> y [t, e] accumulated
    over the core's 2 heads in PSUM.
"""

import numpy as np
import ml_dtypes

B, N, DIM = 2, 2048, 2048
H, HD = 16, 128
T = B * N
NCORES = 8
HPC = H // NCORES  # heads per core
EPS = 1e-5
SCALE = 1.0 / float(np.sqrt(HD))

NB = N // 512     # q-chunks of 512 per batch (4)
KT = N // 128     # k-tiles of 128 per batch (16)
NCH = T // 512    # token chunks of 512 total (8)
NTT = T // 128    # token tiles of 128 total (32)
DO = DIM // 128   # dim tiles (16)
NE = DIM // 512   # out-feature chunks of 512 (4)

_CACHE = {}


def _emit(tc, aps, repeat=1):
    import concourse.bass as bass
    import concourse.mybir as mybir
    from contextlib import ExitStack

    dt = mybir.dt
    f32 = dt.float32
    bf16 = dt.bfloat16
    AF = mybir.ActivationFunctionType
    OP = mybir.AluOpType
    nc = tc.nc

    xt, wq, wk, wv, wo, fr, fi, nw, y = (
        aps["xt"], aps["wq"], aps["wk"], aps["wv"], aps["wo"],
        aps["fr"], aps["fi"], aps["nw"], aps["y"],
    )

    ctx = ExitStack()
    with ctx:
        # ---- persistent SBUF pools ----
        persist = ctx.enter_context(tc.tile_pool(name="persist", bufs=1))
        wq_sb = persist.tile([128, DO, HPC * HD], bf16, tag="wq")
        wk_sb = persist.tile([128, DO, HPC * HD], bf16, tag="wk")
        wv_sb = persist.tile([128, DO, HPC * HD], bf16, tag="wv")
        wo_sb = persist.tile([128, HPC, DIM], bf16, tag="wo")
        nw_sb = persist.tile([128, 2], f32, tag="nw")
        ones_col = persist.tile([128, 1], bf16, tag="ones_col")
        ones_bc = persist.tile([1, 128], f32, tag="ones_bc")
        eps_row = persist.tile([1, 1], f32, tag="eps_row")
        qT = [persist.tile([128, T], bf16, tag=f"qT{h}", name=f"qT{h}")
              for h in range(HPC)]
        kT = [persist.tile([128, T], bf16, tag=f"kT{h}", name=f"kT{h}")
              for h in range(HPC)]
        avn = [persist.tile([128, T], bf16, tag=f"avn{h}", name=f"avn{h}")
               for h in range(HPC)]
        v_sb = persist.tile([128, NTT, HPC * HD], bf16, tag="v_sb")

        for og in range(4):
            osl = slice(og * (DO // 4) * 128, (og + 1) * (DO // 4) * 128)
            otl = slice(og * (DO // 4), (og + 1) * (DO // 4))
            nc.sync.dma_start(
                wq_sb[:, otl, :],
                wq[osl, :].rearrange("(o p) f -> p o f", p=128))
            nc.sync.dma_start(
                wk_sb[:, otl, :],
                wk[osl, :].rearrange("(o p) f -> p o f", p=128))
            nc.sync.dma_start(
                wv_sb[:, otl, :],
                wv[osl, :].rearrange("(o p) f -> p o f", p=128))
        nc.sync.dma_start(wo_sb[:], wo.rearrange("(h p) e -> p h e", p=128))
        nc.sync.dma_start(nw_sb[:], nw[:, :])
        nc.any.memset(ones_col[:], 1.0)
        nc.any.memset(ones_bc[:], 1.0)
        nc.any.memset(eps_row[:], EPS)

        # ---- working pools ----
        xtp = ctx.enter_context(tc.tile_pool(name="xtp", bufs=2))
        frp = ctx.enter_context(tc.tile_pool(name="frp", bufs=2))
        tmp = ctx.enter_context(tc.tile_pool(name="tmp", bufs=2))
        rows = ctx.enter_context(tc.tile_pool(name="rows", bufs=2))
        bcb = ctx.enter_context(tc.tile_pool(name="bcb", bufs=2))
        expp = ctx.enter_context(tc.tile_pool(name="expp", bufs=4))
        ybp = ctx.enter_context(tc.tile_pool(name="ybp", bufs=6))
        # single PSUM pool for all stages; per-tag buffer counts keep the
        # total at 8 banks while letting stages overlap (no pool-scope
        # barriers between qkv / attention / out-projection).
        psum = ctx.enter_context(tc.tile_pool(name="psum", bufs=2, space="PSUM"))

        def stage_a_chunk(ch):
            t0 = ch * 512
            xts = xtp.tile([128, DO, 512], bf16, tag="xts", name="xts")
            for og in range(4):
                osl = slice(og * (DO // 4) * 128, (og + 1) * (DO // 4) * 128)
                otl = slice(og * (DO // 4), (og + 1) * (DO // 4))
                nc.scalar.dma_start(
                    xts[:, otl, :],
                    xt[osl, t0:t0 + 512].rearrange("(o p) t -> p o t", p=128))
            frt = frp.tile([128, 512], f32, tag="frt", name="frt")
            fit = frp.tile([128, 512], f32, tag="fit", name="fit")
            nc.gpsimd.dma_start(frt[:], fr[:, t0:t0 + 512])
            nc.gpsimd.dma_start(fit[:], fi[:, t0:t0 + 512])

            for h in range(HPC):
                hs = h * HD
                q_ps = psum.tile([128, 512], f32, tag="mm", bufs=3, name="q_ps")
                for o in range(DO):
                    nc.tensor.matmul(
                        q_ps[:], wq_sb[:, o, hs:hs + HD], xts[:, o, :],
                        start=(o == 0), stop=(o == DO - 1),
                    )
                q_sbf = tmp.tile([128, 512], f32, tag="q_sbf", name="q_sbf")
                nc.scalar.copy(q_sbf[:], q_ps[:])

                k_ps = psum.tile([128, 512], f32, tag="mm", bufs=3, name="k_ps")
                for o in range(DO):
                    nc.tensor.matmul(
                        k_ps[:], wk_sb[:, o, hs:hs + HD], xts[:, o, :],
                        start=(o == 0), stop=(o == DO - 1),
                    )
                k_sbf = tmp.tile([128, 512], f32, tag="k_sbf", name="k_sbf")
                nc.scalar.copy(k_sbf[:], k_ps[:])

                for src, src_ps, wcol, dstT in (
                        (q_sbf, q_ps, 0, qT[h]), (k_sbf, k_ps, 1, kT[h])):
                    # half-swapped copy (odds to 0:64, evens to 64:128):
                    # ScalarE permits partition-shifted copies
                    qsw = tmp.tile([128, 512], f32, tag="qsw", name="qsw")
                    nc.scalar.copy(qsw[0:64, :], src_ps[64:128, :])
                    nc.scalar.copy(qsw[64:128, :], src_ps[0:64, :])
                    # sum of squares over head dim (partitions) via PE
                    sq = tmp.tile([128, 512], bf16, tag="sq", name="sq")
                    nc.vector.tensor_mul(sq[:], src[:], src[:])
                    ssq = psum.tile([1, 512], f32, tag="row", bufs=2, name="ssq")
                    nc.tensor.matmul(ssq[:], ones_col[:], sq[:],
                                     start=True, stop=True)
                    sqr = rows.tile([1, 512], f32, tag="sqr", name="sqr")
                    nc.scalar.activation(sqr[:], ssq[:], AF.Sqrt,
                                         bias=eps_row[:], scale=1.0 / HD)
                    rsg = rows.tile([1, 512], f32, tag="rsg", name="rsg")
                    nc.vector.reciprocal(rsg[:], sqr[:])
                    # broadcast rsqrt row to 128 partitions on GPSIMD
                    bc = bcb.tile([128, 512], f32, tag="bc", name="bc")
                    nc.gpsimd.partition_broadcast(bc[:], rsg[:])
                    # rope via duplicated-row freqs (all ops aligned):
                    # m1 = [qe*fr; qo*fr], m2 = [qo*fi; qe*fi]
                    # o_top = m1 - m2 (evens), o_bot = m1 + m2 (odds)
                    m1 = tmp.tile([128, 512], f32, tag="m1", name="m1")
                    m2 = tmp.tile([128, 512], f32, tag="m2", name="m2")
                    ot = tmp.tile([128, 512], f32, tag="ot", name="ot")
                    nc.vector.tensor_mul(m1[:], src[:], frt[:])
                    nc.vector.tensor_mul(m2[:], qsw[:], fit[:])
                    nc.vector.tensor_sub(ot[0:64, :], m1[0:64, :], m2[0:64, :])
                    nc.vector.tensor_add(ot[64:128, :], m1[64:128, :],
                                         m2[64:128, :])
                    # normalize: (ot * norm_w) * rsqrt_bc  -> bf16
                    nc.vector.scalar_tensor_tensor(
                        dstT[:, t0:t0 + 512], ot[:],
                        nw_sb[:, wcol:wcol + 1], bc[:],
                        op0=OP.mult, op1=OP.mult,
                    )

            # v projection (natural layout: [t, d]) for both heads
            for tt in range(4):
                v_ps = psum.tile([128, HPC * HD], f32, tag="acc", bufs=3,
                                 name="v_ps")
                for o in range(DO):
                    nc.tensor.matmul(
                        v_ps[:], xts[:, o, tt * 128:tt * 128 + 128],
                        wv_sb[:, o, :],
                        start=(o == 0), stop=(o == DO - 1),
                    )
                nc.scalar.copy(v_sb[:, ch * 4 + tt, :], v_ps[:])

        def stage_b(b, h, qc):
            boff = b * N
            hs = h * HD
            if True:
                qoff = boff + qc * 512
                av_ps = psum.tile([128, 512], f32, tag="acc", bufs=3,
                                  name="av_ps")
                sums = psum.tile([1, 512], f32, tag="row", bufs=2, name="sums")
                for kt in range(KT):
                    koff = boff + kt * 128
                    sc = psum.tile([128, 512], f32, tag="mm", bufs=3, name="sc")
                    nc.tensor.matmul(
                        sc[:], kT[h][:, koff:koff + 128],
                        qT[h][:, qoff:qoff + 512],
                        start=True, stop=True,
                    )
                    ex = expp.tile([128, 512], bf16, tag="ex", name="ex")
                    nc.scalar.activation(ex[:], sc[:], AF.Exp, scale=SCALE)
                    nc.tensor.matmul(
                        av_ps[:], v_sb[:, b * KT + kt, hs:hs + HD], ex[:],
                        start=(kt == 0), stop=(kt == KT - 1),
                    )
                    nc.tensor.matmul(
                        sums[:], ones_col[:], ex[:],
                        start=(kt == 0), stop=(kt == KT - 1),
                    )
                rcp = rows.tile([1, 512], f32, tag="rcp", name="rcp")
                nc.vector.reciprocal(rcp[:], sums[:])
                bc2 = bcb.tile([128, 512], f32, tag="bc2", name="bc2")
                nc.gpsimd.partition_broadcast(bc2[:], rcp[:])
                nc.vector.tensor_mul(avn[h][:, qoff:qoff + 512],
                                     av_ps[:], bc2[:])

        def stage_c(b, tt):
            toff = b * N + tt * 128
            for ec in range(NE):
                eoff = ec * 512
                y_ps = psum.tile([128, 512], f32, tag="acc", bufs=3, name="y_ps")
                for h in range(HPC):
                    nc.tensor.matmul(
                        y_ps[:], avn[h][:, toff:toff + 128],
                        wo_sb[:, h, eoff:eoff + 512],
                        start=(h == 0), stop=(h == HPC - 1),
                    )
                y_sb = ybp.tile([128, 512], bf16, tag="y_sb", name="y_sb")
                if ec % 2 == 0:
                    nc.vector.tensor_copy(y_sb[:], y_ps[:])
                else:
                    nc.scalar.copy(y_sb[:], y_ps[:])
                dma_eng = nc.sync if ec % 2 == 0 else nc.scalar
                dma_eng.dma_start(
                    y[toff:toff + 128, eoff:eoff + 512], y_sb[:]
                )

        for _rep in range(repeat):
            for b in range(B):
                for ch in range(4 * b, 4 * b + 4):
                    stage_a_chunk(ch)
                for h in range(HPC):
                    for qc in range(NB):
                        stage_b(b, h, qc)
                for tt in range(KT):
                    stage_c(b, tt)


def _build(repeat=1):
    key = f"nc{repeat}"
    if key in _CACHE:
        return _CACHE[key]
    import concourse.bacc as bacc
    import concourse.tile as tile
    import concourse.mybir as mybir

    dt = mybir.dt
    nc = bacc.Bacc(
        "TRN2",
        target_bir_lowering=False,
        debug=False,
        enable_asserts=False,
        num_devices=NCORES,
    )
    aps = {
        "xt": nc.dram_tensor("xt", [DIM, T], dt.bfloat16, kind="ExternalInput").ap(),
        "wq": nc.dram_tensor("wq", [DIM, HPC * HD], dt.bfloat16,
                             kind="ExternalInput").ap(),
        "wk": nc.dram_tensor("wk", [DIM, HPC * HD], dt.bfloat16,
                             kind="ExternalInput").ap(),
        "wv": nc.dram_tensor("wv", [DIM, HPC * HD], dt.bfloat16,
                             kind="ExternalInput").ap(),
        "wo": nc.dram_tensor("wo", [HPC * HD, DIM], dt.bfloat16,
                             kind="ExternalInput").ap(),
        "fr": nc.dram_tensor("fr", [128, T], dt.float32, kind="ExternalInput").ap(),
        "fi": nc.dram_tensor("fi", [128, T], dt.float32, kind="ExternalInput").ap(),
        "nw": nc.dram_tensor("nw", [128, 2], dt.float32, kind="ExternalInput").ap(),
        "y": nc.dram_tensor("y", [T, DIM], dt.bfloat16, kind="ExternalOutput").ap(),
    }
    with tile.TileContext(nc) as tc:
        _emit(tc, aps, repeat=repeat)
    nc.compile()
    _CACHE[key] = nc
    return nc


def _prep_inputs(x, freqs_real, freqs_imag, w_qkv, w_out, q_norm_w, k_norm_w):
    bf = ml_dtypes.bfloat16
    x2 = np.asarray(x, np.float32).reshape(T, DIM)
    xt = np.ascontiguousarray(x2.T).astype(bf)
    frT = np.asarray(freqs_real, np.float32).reshape(T, HD // 2).T
    fiT = np.asarray(freqs_imag, np.float32).reshape(T, HD // 2).T
    frT = np.ascontiguousarray(np.concatenate([frT, frT], axis=0))
    fiT = np.ascontiguousarray(np.concatenate([fiT, fiT], axis=0))
    w_qkv = np.asarray(w_qkv, np.float32)
    w_out = np.asarray(w_out, np.float32)
    qnw = np.asarray(q_norm_w, np.float32)
    knw = np.asarray(k_norm_w, np.float32)

    perm = np.concatenate([np.arange(0, HD, 2), np.arange(1, HD, 2)])
    nw = np.stack([qnw[perm], knw[perm]], axis=1).astype(np.float32)
    nw = np.ascontiguousarray(nw)  # [128, 2]

    in_maps = []
    for c in range(NCORES):
        heads = [c * HPC + j for j in range(HPC)]
        wq_c = np.concatenate(
            [w_qkv[:, h * HD:(h + 1) * HD][:, perm] for h in heads], axis=1)
        wk_c = np.concatenate(
            [w_qkv[:, H * HD + h * HD:H * HD + (h + 1) * HD][:, perm]
             for h in heads], axis=1)
        wv_c = np.concatenate(
            [w_qkv[:, 2 * H * HD + h * HD:2 * H * HD + (h + 1) * HD]
             for h in heads], axis=1)
        wo_c = np.concatenate(
            [w_out[h * HD:(h + 1) * HD, :] for h in heads], axis=0)
        in_maps.append({
            "xt": xt,
            "wq": np.ascontiguousarray(wq_c).astype(bf),
            "wk": np.ascontiguousarray(wk_c).astype(bf),
            "wv": np.ascontiguousarray(wv_c).astype(bf),
            "wo": np.ascontiguousarray(wo_c).astype(bf),
            "fr": frT,
            "fi": fiT,
            "nw": nw,
        })
    return in_maps


def kernel(x, freqs_real, freqs_imag, w_qkv, w_out, q_norm_w, k_norm_w,
           _trace=False):
    from concourse.bass_utils import run_bass_kernel_spmd

    nc = _build()
    in_maps = _prep_inputs(x, freqs_real, freqs_imag, w_qkv, w_out,
                           q_norm_w, k_norm_w)
    res = run_bass_kernel_spmd(
        nc, in_maps, core_ids=list(range(NCORES)),
        trace=_trace, trace_cores=[0] if _trace else None,
    )
    _CACHE["last_result"] = res
    out = np.zeros((T, DIM), np.float64)
    for r in res.results:
        out += r["y"].astype(np.float64)
    return out.astype(np.float32).reshape(B, N, DIM)
